# revision 6
# baseline (speedup 1.0000x reference)
"""Bass/Trainium2 kernel for nn_HadamardClassifier.

Math: out = -scale * l2norm(x) @ H + bias, with H = H_16384[:2048, :14951]
(Sylvester). Two structure facts are exploited:
  1. H's columns are 2048-periodic: out[:, j] = Z[:, j %% 2048] + bias[j]
     with Z = xn @ H_2048.
  2. H_2048 = [[A, A], [A, -A]] with A = H_1024, so with
     u = x[:, :1024] + x[:, 1024:], v = x[:, :1024] - x[:, 1024:]:
     Z = [u @ A | v @ A] -- one FWHT butterfly level. Only the 2 MB A is
     loaded from HBM and matmul MACs halve.

Sharding: batch-parallel across 8 cores (512 rows each).

Numerics (gate is rel_err < 2e-2 vs max|out|): A entries are exactly +-1 so
bf16 is lossless; u/v are rounded to bf16 (f32 PSUM accumulate) ~1e-3 rel
err; the row factor -scale/||x|| is applied per-partition during the
PSUM->SBUF copy; output is stored as fp16 (~3e-4 additional) and upcast to
f32 on the host after the gather. Total ~1.6e-3.

Engine plan per core: sync/HWDGE does all HBM traffic (loads up front, then
one [128,7,512] out-DMA + tail per (js, cb)); DVE does butterflies and the
bias adds as one stride-0-broadcast [128, nblk, 512] op per (js, cb); PE
transposes u/v (bf16) and streams 128 matmuls; ACT does norms + scaled
PSUM->fp16 copies; GpSimd only does 4 wide bias partition-broadcasts.
js0's compute is interleaved into phase 1 so the out-stream starts early.
"""

import math

import numpy as np

B, IN, OUT = 4096, 2048, 14951
NCORES = 8
BLOC = B // NCORES  # 512
P = 128
PERIOD = 2048
HALF = 1024
NFULL = 7  # full 2048-wide output blocks
TAIL = OUT - NFULL * PERIOD  # 615
EPS = 1e-12
NCB = BLOC // P  # 4 batch chunks per core
NIC = HALF // P  # 8 contraction chunks (over A's rows)
NJS = PERIOD // 512  # 4 column slabs of 512
# bias packed js-major: per js, its 512-col piece of every block, 512-padded
NBLK_JS = [8, 8, 7, 7]  # js1's 8th seg is the 103-wide tail (padded)
OFF_JS = [0, 4096, 8192, 11776]
BIAS_PACK = 15360
TAILW = [512, 103, 0, 0]  # tail-block width per js

_CACHE = {}
LAST_RESULT = None
PROFILE = False


def _build(scale_val: float):
    from contextlib import ExitStack

    import concourse.bass as bass
    import concourse.mybir as mybir
    import concourse.tile as tile
    from concourse import bacc, masks

    f32 = mybir.dt.float32
    bf16 = mybir.dt.bfloat16
    f16 = mybir.dt.float16
    nc = bacc.Bacc("TRN2", target_bir_lowering=False, debug=False,
                   num_devices=NCORES)

    x_d = nc.dram_tensor("x", [BLOC, IN], f32, kind="ExternalInput")
    h_d = nc.dram_tensor("h", [HALF, HALF], bf16, kind="ExternalInput")
    b_d = nc.dram_tensor("bias", [1, BIAS_PACK], f16, kind="ExternalInput")
    o_d = nc.dram_tensor("out", [BLOC, OUT], f16, kind="ExternalOutput")

    # [1024 rows] -> [p, ic] view so each SBUF partition p holds rows ic*128+p
    h_v = h_d[:, :].rearrange("(ic p) j -> p ic j", p=P)
    # blocks 0..6 of the output as [rows, blk, col-in-block]
    o_main = o_d[:, 0 : NFULL * PERIOD].rearrange("r (blk c) -> r blk c",
                                                  c=PERIOD)

    with tile.TileContext(nc) as tc, ExitStack() as ctx:
        p_const = ctx.enter_context(tc.tile_pool(name="const", bufs=1))
        p_x = ctx.enter_context(tc.tile_pool(name="xload", bufs=NCB))
        p_uv = ctx.enter_context(tc.tile_pool(name="uv", bufs=2))
        p_w = ctx.enter_context(tc.tile_pool(name="work", bufs=1))
        p_ss = ctx.enter_context(tc.tile_pool(name="small", bufs=16))
        p_xth = ctx.enter_context(tc.tile_pool(name="xth", bufs=NCB))
        p_h = ctx.enter_context(tc.tile_pool(name="hslab", bufs=2))
        p_z = ctx.enter_context(tc.tile_pool(name="zsb", bufs=4))
        p_o = ctx.enter_context(tc.tile_pool(name="ostage", bufs=3))
        p_pst = ctx.enter_context(
            tc.tile_pool(name="psum_t", bufs=2, space="PSUM"))
        p_psw = ctx.enter_context(
            tc.tile_pool(name="psum_w", bufs=1, space="PSUM"))
        p_psz = ctx.enter_context(
            tc.tile_pool(name="psum_z", bufs=4, space="PSUM"))

        identb = p_const.tile([P, P], bf16, tag="identb")
        masks.make_identity(nc, identb[:])

        # HAM warmup: keep the PE busy early so the clock gate opens
        # (4/8 -> 8/8) before the real matmul stream starts
        warm = p_psw.tile([P, P], bf16, tag="warm")
        for _ in range(16):
            nc.tensor.transpose(warm[:], identb[:], identb[:])

        # ---- all HBM loads up front on the sync/HWDGE queue, in priority
        # order: bias row, x chunks with A's halves interleaved
        bias_rep = p_const.tile([P, BIAS_PACK], f16, tag="bias_rep")
        nc.sync.dma_start(out=bias_rep[0:1, :], in_=b_d[:, :])

        xnats = []
        hq_tiles = {}

        def load_h(half):
            hq = p_h.tile([P, NIC, 512], bf16, tag="hslab")
            nc.sync.dma_start(
                out=hq[:], in_=h_v[:, :, half * 512 : half * 512 + 512])
            hq_tiles[half] = hq

        for cb in range(NCB):
            xnat = p_x.tile([P, IN], f32, tag="xnat")
            nc.sync.dma_start(out=xnat[:], in_=x_d[cb * P : (cb + 1) * P, :])
            xnats.append(xnat)
            if cb < 2:
                load_h(cb)

        # ---- bias replication: 4 wide partition-broadcasts on gpsimd
        # (its queue is otherwise idle), js0's segment first
        for js in range(NJS):
            o0 = OFF_JS[js]
            w = NBLK_JS[js] * 512
            nc.gpsimd.partition_broadcast(bias_rep[:, o0 : o0 + w],
                                          bias_rep[0:1, o0 : o0 + w])

        mults = []
        xths = []

        def phase1(cb):
            xnat = xnats[cb]
            sq = p_w.tile([P, IN], bf16, tag="work")
            ss = p_ss.tile([P, 1], f32, tag="ss")
            nc.scalar.activation(sq[:], xnat[:],
                                 mybir.ActivationFunctionType.Square,
                                 accum_out=ss[:])
            nc.vector.tensor_scalar_max(ss[:], ss[:], EPS)
            nrm = p_ss.tile([P, 1], f32, tag="nrm")
            nc.scalar.sqrt(nrm[:], ss[:])
            inv = p_ss.tile([P, 1], f32, tag="inv")
            nc.vector.reciprocal(inv[:], nrm[:])
            mult = p_ss.tile([P, 1], f32, tag="mult")
            nc.vector.tensor_scalar_mul(mult[:], inv[:], -scale_val)
            mults.append(mult)

            # FWHT butterfly level on DVE: u | v, rounded to bf16
            uv = p_uv.tile([P, 2, HALF], bf16, tag="uv")
            nc.vector.tensor_add(uv[:, 0, :], xnat[:, 0:HALF],
                                 xnat[:, HALF:IN])
            nc.vector.tensor_sub(uv[:, 1, :], xnat[:, 0:HALF],
                                 xnat[:, HALF:IN])

            # transpose u/v chunks on PE (bf16 in -> bf16 PSUM); copies to
            # SBUF split between ACT and DVE to keep both pipelines short
            xth = p_xth.tile([P, 2, NIC, P], bf16, tag="xth")
            for g in range(2):
                for ic in range(NIC):
                    pst = p_pst.tile([P, P], bf16, tag="pst")
                    nc.tensor.transpose(
                        pst[:], uv[:, g, ic * P : (ic + 1) * P], identb[:])
                    if ic % 2 == 0:
                        nc.scalar.copy(xth[:, g, ic, :], pst[:])
                    else:
                        nc.vector.tensor_copy(xth[:, g, ic, :], pst[:])
            xths.append(xth)

        def phase2(js, cb):
            c0 = js * 512
            boff = OFF_JS[js]
            nblk = NBLK_JS[js]
            bseg = bias_rep[:, boff : boff + nblk * 512].rearrange(
                "p (b c) -> p b c", c=512)
            g, half = js // 2, js % 2
            psz = p_psz.tile([P, 512], f32, tag="psz")
            for ic in range(NIC):
                nc.tensor.matmul(psz[:], xths[cb][:, g, ic, :],
                                 hq_tiles[half][:, ic, :],
                                 start=(ic == 0), stop=(ic == NIC - 1))
            # psz * (-scale/||x||) -> fp16, per-partition scale on ACT
            zsb = p_z.tile([P, 1, 512], f16, tag="zsb")
            nc.scalar.mul(zsb[:, 0, :], psz[:], mults[cb][:, 0:1])

            ost = p_o.tile([P, 8, 512], f16, tag="ostage")
            nc.vector.tensor_add(
                ost[:, 0:nblk, :], zsb[:].to_broadcast([P, nblk, 512]), bseg)

            r0 = cb * P
            nc.sync.dma_start(
                out=o_main[r0 : r0 + P, 0:NFULL, c0 : c0 + 512],
                in_=ost[:, 0:NFULL, :])
            tw = TAILW[js]
            if tw:
                nc.sync.dma_start(
                    out=o_d[r0 : r0 + P, NFULL * PERIOD + c0 :
                            NFULL * PERIOD + c0 + tw],
                    in_=ost[:, NFULL, 0:tw])

        # software pipeline: js0's iteration for chunk cb-1 is emitted
        # between phase-1 chunks so the out-stream starts early
        for cb in range(NCB):
            phase1(cb)
            if cb >= 1:
                phase2(0, cb - 1)
        phase2(0, NCB - 1)
        for js in range(1, NJS):
            for cb in range(NCB):
                phase2(js, cb)

    nc.compile()
    return nc


def _pack_bias(bias: np.ndarray) -> np.ndarray:
    pack = np.zeros((1, BIAS_PACK), dtype=np.float16)
    for js in range(NJS):
        for blk in range(NBLK_JS[js]):
            src0 = blk * PERIOD + js * 512
            seg = bias[src0 : src0 + 512]
            pack[0, OFF_JS[js] + blk * 512 : OFF_JS[js] + blk * 512 + len(seg)] = seg
    return pack


def kernel(x, hadamard, scale, bias):
    global LAST_RESULT
    import ml_dtypes
    from concourse.bass_utils import run_bass_kernel_spmd

    x = np.ascontiguousarray(np.asarray(x, dtype=np.float32))
    hadamard = np.asarray(hadamard, dtype=np.float32)
    bias = np.asarray(bias, dtype=np.float32)
    scale_val = float(np.asarray(scale).reshape(-1)[0])

    h2 = np.ascontiguousarray(hadamard[:, :PERIOD])
    # the whole kernel rests on the 2048-periodicity of the weight columns
    for k in range(1, NFULL):
        assert np.array_equal(hadamard[:, k * PERIOD : (k + 1) * PERIOD], h2), (
            "hadamard is not 2048-periodic; kernel assumption violated")
    assert np.array_equal(hadamard[:, NFULL * PERIOD :], h2[:, :TAIL])
    # ... and on the Sylvester block structure H_2048 = [[A, A], [A, -A]]
    A = h2[:HALF, :HALF]
    assert np.array_equal(h2[HALF:, :HALF], A)
    assert np.array_equal(h2[:HALF, HALF:], A)
    assert np.array_equal(h2[HALF:, HALF:], -A)
    Ab = A.astype(ml_dtypes.bfloat16)
    assert np.array_equal(Ab.astype(np.float32), A), "A not bf16-exact"

    key = scale_val
    if key not in _CACHE:
        _CACHE[key] = _build(scale_val)
    nc = _CACHE[key]

    bias_pack = _pack_bias(bias)
    in_maps = [
        {"x": np.ascontiguousarray(x[c * BLOC : (c + 1) * BLOC]),
         "h": Ab, "bias": bias_pack}
        for c in range(NCORES)
    ]
    res = run_bass_kernel_spmd(nc, in_maps, list(range(NCORES)),
                               trace=PROFILE)
    LAST_RESULT = res
    out = np.concatenate([res.results[c]["out"] for c in range(NCORES)],
                         axis=0).astype(np.float32)
    return out


# revision 7
# speedup vs baseline: 1.0915x; 1.0915x over previous
"""Bass/Trainium2 kernel for nn_HadamardClassifier.

Math: out = -scale * l2norm(x) @ H + bias, with H = H_16384[:2048, :14951]
(Sylvester). Two structure facts are exploited:
  1. H's columns are 2048-periodic: out[:, j] = Z[:, j %% 2048] + bias[j]
     with Z = xn @ H_2048.
  2. H_2048 = [[A, A], [A, -A]] with A = H_1024, so with
     u = x[:, :1024] + x[:, 1024:], v = x[:, :1024] - x[:, 1024:]:
     Z = [u @ A | v @ A] -- one FWHT butterfly level. Only the 2 MB A is
     loaded from HBM and matmul MACs halve.

Sharding: batch-parallel across 8 cores (512 rows each).

Numerics (gate is rel_err < 2e-2 vs max|out|): A entries are exactly +-1 so
bf16 is lossless; u/v are rounded to bf16 (f32 PSUM accumulate) ~1e-3 rel
err; the row factor -scale/||x|| is applied per-partition during the
PSUM->SBUF copy; output is stored as fp16 (~3e-4 additional) and upcast to
f32 on the host after the gather. Total ~1.6e-3.

Engine plan per core: sync/HWDGE does all HBM traffic (loads up front, then
one [128,7,512] out-DMA + tail per (js, cb)); DVE does butterflies and the
bias adds as one stride-0-broadcast [128, nblk, 512] op per (js, cb); PE
transposes u/v (bf16) and streams 128 matmuls; ACT does norms + scaled
PSUM->fp16 copies; GpSimd only does 4 wide bias partition-broadcasts.
js0's compute is interleaved into phase 1 so the out-stream starts early.
"""

import math

import numpy as np

B, IN, OUT = 4096, 2048, 14951
NCORES = 8
BLOC = B // NCORES  # 512
P = 128
PERIOD = 2048
HALF = 1024
NFULL = 7  # full 2048-wide output blocks
TAIL = OUT - NFULL * PERIOD  # 615
EPS = 1e-12
NCB = BLOC // P  # 4 batch chunks per core
NIC = HALF // P  # 8 contraction chunks (over A's rows)
NJS = PERIOD // 512  # 4 column slabs of 512
# bias packed js-major: per js, its 512-col piece of every block, 512-padded
NBLK_JS = [8, 8, 7, 7]  # js1's 8th seg is the 103-wide tail (padded)
OFF_JS = [0, 4096, 8192, 11776]
BIAS_PACK = 15360
TAILW = [512, 103, 0, 0]  # tail-block width per js

_CACHE = {}
LAST_RESULT = None
PROFILE = False


def _build(scale_val: float):
    from contextlib import ExitStack

    import concourse.bass as bass
    import concourse.mybir as mybir
    import concourse.tile as tile
    from concourse import bacc, masks

    f32 = mybir.dt.float32
    bf16 = mybir.dt.bfloat16
    f16 = mybir.dt.float16
    nc = bacc.Bacc("TRN2", target_bir_lowering=False, debug=False,
                   num_devices=NCORES)

    x_d = nc.dram_tensor("x", [BLOC, IN], f32, kind="ExternalInput")
    h_d = nc.dram_tensor("h", [HALF, HALF], bf16, kind="ExternalInput")
    b_d = nc.dram_tensor("bias", [1, BIAS_PACK], f16, kind="ExternalInput")
    o_d = nc.dram_tensor("out", [BLOC, OUT], f16, kind="ExternalOutput")

    # [1024 rows] -> [p, ic] view so each SBUF partition p holds rows ic*128+p
    h_v = h_d[:, :].rearrange("(ic p) j -> p ic j", p=P)
    # blocks 0..6 of the output as [rows, blk, col-in-block]
    o_main = o_d[:, 0 : NFULL * PERIOD].rearrange("r (blk c) -> r blk c",
                                                  c=PERIOD)

    with tile.TileContext(nc) as tc, ExitStack() as ctx:
        p_const = ctx.enter_context(tc.tile_pool(name="const", bufs=1))
        p_x = ctx.enter_context(tc.tile_pool(name="xload", bufs=NCB))
        p_uv = ctx.enter_context(tc.tile_pool(name="uv", bufs=2))
        p_w = ctx.enter_context(tc.tile_pool(name="work", bufs=1))
        p_ss = ctx.enter_context(tc.tile_pool(name="small", bufs=16))
        p_xth = ctx.enter_context(tc.tile_pool(name="xth", bufs=NCB))
        p_h = ctx.enter_context(tc.tile_pool(name="hslab", bufs=2))
        p_z = ctx.enter_context(tc.tile_pool(name="zsb", bufs=4))
        p_o = ctx.enter_context(tc.tile_pool(name="ostage", bufs=3))
        p_pst = ctx.enter_context(
            tc.tile_pool(name="psum_t", bufs=2, space="PSUM"))
        p_psw = ctx.enter_context(
            tc.tile_pool(name="psum_w", bufs=1, space="PSUM"))
        p_psz = ctx.enter_context(
            tc.tile_pool(name="psum_z", bufs=4, space="PSUM"))

        identb = p_const.tile([P, P], bf16, tag="identb")
        masks.make_identity(nc, identb[:])

        # HAM warmup: keep the PE busy early so the clock gate opens
        # (4/8 -> 8/8) before the real matmul stream starts
        warm = p_psw.tile([P, P], bf16, tag="warm")
        for _ in range(16):
            nc.tensor.transpose(warm[:], identb[:], identb[:])

        # ---- all HBM loads up front on the sync/HWDGE queue, in priority
        # order: bias row, x chunks with A's halves interleaved
        bias_rep = p_const.tile([P, BIAS_PACK], f16, tag="bias_rep")
        nc.sync.dma_start(out=bias_rep[0:1, :], in_=b_d[:, :])

        xnats = []
        hq_tiles = {}

        def load_h(half):
            hq = p_h.tile([P, NIC, 512], bf16, tag="hslab")
            nc.sync.dma_start(
                out=hq[:], in_=h_v[:, :, half * 512 : half * 512 + 512])
            hq_tiles[half] = hq

        for cb in range(NCB):
            xnat = p_x.tile([P, IN], f32, tag="xnat")
            nc.sync.dma_start(out=xnat[:], in_=x_d[cb * P : (cb + 1) * P, :])
            xnats.append(xnat)
            if cb < 2:
                load_h(cb)

        # ---- bias replication on gpsimd (its queue is otherwise idle) in
        # 512-wide chunks, js0's blocks first: the js0 adds are interleaved
        # into phase 1, so js0's segment must be ready early (~1us/chunk;
        # one wide op per js measured 8us and stalled the js0 adds)
        for js in range(NJS):
            o0 = OFF_JS[js]
            for blk in range(NBLK_JS[js]):
                a = o0 + blk * 512
                nc.gpsimd.partition_broadcast(bias_rep[:, a : a + 512],
                                              bias_rep[0:1, a : a + 512])

        mults = []
        xths = []

        def phase1(cb):
            xnat = xnats[cb]
            sq = p_w.tile([P, IN], bf16, tag="work")
            ss = p_ss.tile([P, 1], f32, tag="ss")
            nc.scalar.activation(sq[:], xnat[:],
                                 mybir.ActivationFunctionType.Square,
                                 accum_out=ss[:])
            nc.vector.tensor_scalar_max(ss[:], ss[:], EPS)
            nrm = p_ss.tile([P, 1], f32, tag="nrm")
            nc.scalar.sqrt(nrm[:], ss[:])
            inv = p_ss.tile([P, 1], f32, tag="inv")
            nc.vector.reciprocal(inv[:], nrm[:])
            mult = p_ss.tile([P, 1], f32, tag="mult")
            nc.vector.tensor_scalar_mul(mult[:], inv[:], -scale_val)
            mults.append(mult)

            # FWHT butterfly level on DVE: u | v, rounded to bf16
            uv = p_uv.tile([P, 2, HALF], bf16, tag="uv")
            nc.vector.tensor_add(uv[:, 0, :], xnat[:, 0:HALF],
                                 xnat[:, HALF:IN])
            nc.vector.tensor_sub(uv[:, 1, :], xnat[:, 0:HALF],
                                 xnat[:, HALF:IN])

            # transpose u/v chunks on PE (bf16 in -> bf16 PSUM); copies to
            # SBUF split between ACT and DVE to keep both pipelines short
            xth = p_xth.tile([P, 2, NIC, P], bf16, tag="xth")
            for g in range(2):
                for ic in range(NIC):
                    pst = p_pst.tile([P, P], bf16, tag="pst")
                    nc.tensor.transpose(
                        pst[:], uv[:, g, ic * P : (ic + 1) * P], identb[:])
                    if ic % 2 == 0:
                        nc.scalar.copy(xth[:, g, ic, :], pst[:])
                    else:
                        nc.vector.tensor_copy(xth[:, g, ic, :], pst[:])
            xths.append(xth)

        def phase2(js, cb):
            c0 = js * 512
            boff = OFF_JS[js]
            nblk = NBLK_JS[js]
            bseg = bias_rep[:, boff : boff + nblk * 512].rearrange(
                "p (b c) -> p b c", c=512)
            g, half = js // 2, js % 2
            psz = p_psz.tile([P, 512], f32, tag="psz")
            for ic in range(NIC):
                nc.tensor.matmul(psz[:], xths[cb][:, g, ic, :],
                                 hq_tiles[half][:, ic, :],
                                 start=(ic == 0), stop=(ic == NIC - 1))
            # psz * (-scale/||x||) -> fp16, per-partition scale on ACT
            zsb = p_z.tile([P, 1, 512], f16, tag="zsb")
            nc.scalar.mul(zsb[:, 0, :], psz[:], mults[cb][:, 0:1])

            ost = p_o.tile([P, 8, 512], f16, tag="ostage")
            nc.vector.tensor_add(
                ost[:, 0:nblk, :], zsb[:].to_broadcast([P, nblk, 512]), bseg)

            r0 = cb * P
            nc.sync.dma_start(
                out=o_main[r0 : r0 + P, 0:NFULL, c0 : c0 + 512],
                in_=ost[:, 0:NFULL, :])
            tw = TAILW[js]
            if tw:
                nc.sync.dma_start(
                    out=o_d[r0 : r0 + P, NFULL * PERIOD + c0 :
                            NFULL * PERIOD + c0 + tw],
                    in_=ost[:, NFULL, 0:tw])

        # software pipeline: js0's iteration for chunk cb-1 is emitted
        # between phase-1 chunks so the out-stream starts early
        for cb in range(NCB):
            phase1(cb)
            if cb >= 1:
                phase2(0, cb - 1)
        phase2(0, NCB - 1)
        for js in range(1, NJS):
            for cb in range(NCB):
                phase2(js, cb)

    nc.compile()
    return nc


def _pack_bias(bias: np.ndarray) -> np.ndarray:
    pack = np.zeros((1, BIAS_PACK), dtype=np.float16)
    for js in range(NJS):
        for blk in range(NBLK_JS[js]):
            src0 = blk * PERIOD + js * 512
            seg = bias[src0 : src0 + 512]
            pack[0, OFF_JS[js] + blk * 512 : OFF_JS[js] + blk * 512 + len(seg)] = seg
    return pack


def kernel(x, hadamard, scale, bias):
    global LAST_RESULT
    import ml_dtypes
    from concourse.bass_utils import run_bass_kernel_spmd

    x = np.ascontiguousarray(np.asarray(x, dtype=np.float32))
    hadamard = np.asarray(hadamard, dtype=np.float32)
    bias = np.asarray(bias, dtype=np.float32)
    scale_val = float(np.asarray(scale).reshape(-1)[0])

    h2 = np.ascontiguousarray(hadamard[:, :PERIOD])
    # the whole kernel rests on the 2048-periodicity of the weight columns
    for k in range(1, NFULL):
        assert np.array_equal(hadamard[:, k * PERIOD : (k + 1) * PERIOD], h2), (
            "hadamard is not 2048-periodic; kernel assumption violated")
    assert np.array_equal(hadamard[:, NFULL * PERIOD :], h2[:, :TAIL])
    # ... and on the Sylvester block structure H_2048 = [[A, A], [A, -A]]
    A = h2[:HALF, :HALF]
    assert np.array_equal(h2[HALF:, :HALF], A)
    assert np.array_equal(h2[:HALF, HALF:], A)
    assert np.array_equal(h2[HALF:, HALF:], -A)
    Ab = A.astype(ml_dtypes.bfloat16)
    assert np.array_equal(Ab.astype(np.float32), A), "A not bf16-exact"

    key = scale_val
    if key not in _CACHE:
        _CACHE[key] = _build(scale_val)
    nc = _CACHE[key]

    bias_pack = _pack_bias(bias)
    in_maps = [
        {"x": np.ascontiguousarray(x[c * BLOC : (c + 1) * BLOC]),
         "h": Ab, "bias": bias_pack}
        for c in range(NCORES)
    ]
    res = run_bass_kernel_spmd(nc, in_maps, list(range(NCORES)),
                               trace=PROFILE)
    LAST_RESULT = res
    out = np.concatenate([res.results[c]["out"] for c in range(NCORES)],
                         axis=0).astype(np.float32)
    return out


# revision 10
# speedup vs baseline: 1.1954x; 1.0952x over previous
"""Bass/Trainium2 kernel for nn_HadamardClassifier.

Math: out = -scale * l2norm(x) @ H + bias, with H = H_16384[:2048, :14951]
(Sylvester). Two structure facts are exploited:
  1. H's columns are 2048-periodic: out[:, j] = Z[:, j %% 2048] + bias[j]
     with Z = xn @ H_2048.
  2. H_2048 = [[A, A], [A, -A]] with A = H_1024, so with
     u = x[:, :1024] + x[:, 1024:], v = x[:, :1024] - x[:, 1024:]:
     Z = [u @ A | v @ A] -- one FWHT butterfly level. Only the 2 MB A is
     loaded from HBM and matmul MACs halve.

Sharding: batch-parallel across 8 cores (512 rows each).

Numerics (gate is rel_err < 2e-2 vs max|out|): A entries are exactly +-1 so
bf16 is lossless; u/v are rounded to bf16 (f32 PSUM accumulate) ~1e-3 rel
err; the row factor -scale/||x|| is applied per-partition during the
PSUM->SBUF copy; output is stored as fp16 (~3e-4 additional) and upcast to
f32 on the host after the gather. Total ~1.6e-3.

Engine plan per core: sync/HWDGE does all HBM traffic (loads up front, then
one [128,7,512] out-DMA + tail per (js, cb)); DVE does butterflies and the
bias adds as one stride-0-broadcast [128, nblk, 512] op per (js, cb); PE
transposes u/v (bf16) and streams 128 matmuls; ACT does norms + scaled
PSUM->fp16 copies; GpSimd only does 4 wide bias partition-broadcasts.
js0's compute is interleaved into phase 1 so the out-stream starts early.
"""

import math

import numpy as np

B, IN, OUT = 4096, 2048, 14951
NCORES = 8
BLOC = B // NCORES  # 512
P = 128
PERIOD = 2048
HALF = 1024
NFULL = 7  # full 2048-wide output blocks
TAIL = OUT - NFULL * PERIOD  # 615
EPS = 1e-12
NCB = BLOC // P  # 4 batch chunks per core
NIC = HALF // P  # 8 contraction chunks (over A's rows)
NJS = PERIOD // 512  # 4 column slabs of 512
# bias packed js-major: per js, its 512-col piece of every block, 512-padded
NBLK_JS = [8, 8, 7, 7]  # js1's 8th seg is the 103-wide tail (padded)
OFF_JS = [0, 4096, 8192, 11776]
BIAS_PACK = 15360
TAILW = [512, 103, 0, 0]  # tail-block width per js

_CACHE = {}
LAST_RESULT = None
PROFILE = False


def _build(scale_val: float):
    from contextlib import ExitStack

    import concourse.bass as bass
    import concourse.mybir as mybir
    import concourse.tile as tile
    from concourse import bacc, masks

    f32 = mybir.dt.float32
    bf16 = mybir.dt.bfloat16
    f16 = mybir.dt.float16
    nc = bacc.Bacc("TRN2", target_bir_lowering=False, debug=False,
                   num_devices=NCORES)

    x_d = nc.dram_tensor("x", [BLOC, IN], f32, kind="ExternalInput")
    h_d = nc.dram_tensor("h", [HALF, HALF], bf16, kind="ExternalInput")
    b_d = nc.dram_tensor("bias", [1, BIAS_PACK], f16, kind="ExternalInput")
    o_d = nc.dram_tensor("out", [BLOC, OUT], f16, kind="ExternalOutput")

    # [1024 rows] -> [p, ic] view so each SBUF partition p holds rows ic*128+p
    h_v = h_d[:, :].rearrange("(ic p) j -> p ic j", p=P)
    # blocks 0..6 of the output as [rows, blk, col-in-block]
    o_main = o_d[:, 0 : NFULL * PERIOD].rearrange("r (blk c) -> r blk c",
                                                  c=PERIOD)

    with tile.TileContext(nc) as tc, ExitStack() as ctx:
        p_const = ctx.enter_context(tc.tile_pool(name="const", bufs=1))
        p_x = ctx.enter_context(tc.tile_pool(name="xload", bufs=NCB))
        p_uv = ctx.enter_context(tc.tile_pool(name="uv", bufs=2))
        p_w = ctx.enter_context(tc.tile_pool(name="work", bufs=1))
        p_ss = ctx.enter_context(tc.tile_pool(name="small", bufs=16))
        p_xth = ctx.enter_context(tc.tile_pool(name="xth", bufs=NCB))
        p_h = ctx.enter_context(tc.tile_pool(name="hslab", bufs=2))
        p_z = ctx.enter_context(tc.tile_pool(name="zsb", bufs=4))
        p_o = ctx.enter_context(tc.tile_pool(name="ostage", bufs=3))
        p_pst = ctx.enter_context(
            tc.tile_pool(name="psum_t", bufs=2, space="PSUM"))
        p_psw = ctx.enter_context(
            tc.tile_pool(name="psum_w", bufs=1, space="PSUM"))
        p_psz = ctx.enter_context(
            tc.tile_pool(name="psum_z", bufs=4, space="PSUM"))

        identb = p_const.tile([P, P], bf16, tag="identb")
        masks.make_identity(nc, identb[:])

        # HAM management: the NC clock gate throttles to 4/8 whenever the PE
        # sees a fully-idle ~3.4us window, and the throttle slows EVERY
        # engine (DMA descriptors, DVE, Q7 all measured ~1.9x slower when
        # cold). The FWHT split leaves the PE mostly idle, so dummy PE ops
        # (identity transposes / junk matmuls into a scratch PSUM bank) are
        # threaded through the whole kernel to keep the gate open.
        warmT = p_psw.tile([P, P], bf16, tag="warmT")

        def keep_warm_t(n):
            for _ in range(n):
                nc.tensor.transpose(warmT[:], identb[:], identb[:])

        # cold-start: ~3.4us of sustained PE activity opens the gate
        keep_warm_t(16)

        # ---- all HBM loads up front on the sync/HWDGE queue, in priority
        # order: bias row, x chunks with A's halves interleaved
        bias_rep = p_const.tile([P, BIAS_PACK], f16, tag="bias_rep")
        nc.sync.dma_start(out=bias_rep[0:1, :], in_=b_d[:, :])

        xnats = []
        hq_tiles = {}

        def load_h(half):
            hq = p_h.tile([P, NIC, 512], bf16, tag="hslab")
            nc.sync.dma_start(
                out=hq[:], in_=h_v[:, :, half * 512 : half * 512 + 512])
            hq_tiles[half] = hq

        for cb in range(NCB):
            xnat = p_x.tile([P, IN], f32, tag="xnat")
            nc.sync.dma_start(out=xnat[:], in_=x_d[cb * P : (cb + 1) * P, :])
            xnats.append(xnat)
            if cb < 2:
                load_h(cb)

        # ---- bias replication on gpsimd (its queue is otherwise idle) in
        # 512-wide chunks, js0's blocks first: the js0 adds are interleaved
        # into phase 1, so js0's segment must be ready early (~1us/chunk;
        # one wide op per js measured 8us and stalled the js0 adds)
        for js in range(NJS):
            o0 = OFF_JS[js]
            for blk in range(NBLK_JS[js]):
                a = o0 + blk * 512
                nc.gpsimd.partition_broadcast(bias_rep[:, a : a + 512],
                                              bias_rep[0:1, a : a + 512])

        warmM = p_psw.tile([P, 512], f32, tag="warmM")

        def keep_warm_m(n):
            for _ in range(n):
                nc.tensor.matmul(warmM[:], identb[:], hq_tiles[0][:, 0, :],
                                 start=True, stop=True)

        mults = []
        xths = []

        def phase1(cb):
            xnat = xnats[cb]
            sq = p_w.tile([P, IN], bf16, tag="work")
            ss = p_ss.tile([P, 1], f32, tag="ss")
            nc.scalar.activation(sq[:], xnat[:],
                                 mybir.ActivationFunctionType.Square,
                                 accum_out=ss[:])
            nc.vector.tensor_scalar_max(ss[:], ss[:], EPS)
            nrm = p_ss.tile([P, 1], f32, tag="nrm")
            nc.scalar.sqrt(nrm[:], ss[:])
            inv = p_ss.tile([P, 1], f32, tag="inv")
            nc.vector.reciprocal(inv[:], nrm[:])
            mult = p_ss.tile([P, 1], f32, tag="mult")
            nc.vector.tensor_scalar_mul(mult[:], inv[:], -scale_val)
            mults.append(mult)

            # FWHT butterfly level on DVE: u | v, rounded to bf16
            uv = p_uv.tile([P, 2, HALF], bf16, tag="uv")
            nc.vector.tensor_add(uv[:, 0, :], xnat[:, 0:HALF],
                                 xnat[:, HALF:IN])
            nc.vector.tensor_sub(uv[:, 1, :], xnat[:, 0:HALF],
                                 xnat[:, HALF:IN])

            # transpose u/v chunks on PE (bf16 in -> bf16 PSUM); copies to
            # SBUF split between ACT and DVE to keep both pipelines short
            xth = p_xth.tile([P, 2, NIC, P], bf16, tag="xth")
            for g in range(2):
                for ic in range(NIC):
                    pst = p_pst.tile([P, P], bf16, tag="pst")
                    nc.tensor.transpose(
                        pst[:], uv[:, g, ic * P : (ic + 1) * P], identb[:])
                    if ic % 2 == 0:
                        nc.scalar.copy(xth[:, g, ic, :], pst[:])
                    else:
                        nc.vector.tensor_copy(xth[:, g, ic, :], pst[:])
            xths.append(xth)

        def phase2(js, cb):
            c0 = js * 512
            boff = OFF_JS[js]
            nblk = NBLK_JS[js]
            bseg = bias_rep[:, boff : boff + nblk * 512].rearrange(
                "p (b c) -> p b c", c=512)
            g, half = js // 2, js % 2
            psz = p_psz.tile([P, 512], f32, tag="psz")
            for ic in range(NIC):
                nc.tensor.matmul(psz[:], xths[cb][:, g, ic, :],
                                 hq_tiles[half][:, ic, :],
                                 start=(ic == 0), stop=(ic == NIC - 1))
            # psz * (-scale/||x||) -> fp16, per-partition scale on ACT
            zsb = p_z.tile([P, 1, 512], f16, tag="zsb")
            nc.scalar.mul(zsb[:, 0, :], psz[:], mults[cb][:, 0:1])

            ost = p_o.tile([P, 8, 512], f16, tag="ostage")
            nc.vector.tensor_add(
                ost[:, 0:nblk, :], zsb[:].to_broadcast([P, nblk, 512]), bseg)

            r0 = cb * P
            nc.sync.dma_start(
                out=o_main[r0 : r0 + P, 0:NFULL, c0 : c0 + 512],
                in_=ost[:, 0:NFULL, :])
            tw = TAILW[js]
            if tw:
                nc.sync.dma_start(
                    out=o_d[r0 : r0 + P, NFULL * PERIOD + c0 :
                            NFULL * PERIOD + c0 + tw],
                    in_=ost[:, NFULL, 0:tw])

        # software pipeline: js0's iteration for chunk cb-1 is emitted
        # between phase-1 chunks so the out-stream starts early; keep-warm
        # fillers bridge every PE wait so the clock gate never closes
        keep_warm_t(24)
        for cb in range(NCB):
            phase1(cb)
            keep_warm_t(16)
            if cb >= 1:
                phase2(0, cb - 1)
                keep_warm_m(3)
        phase2(0, NCB - 1)
        for js in range(1, NJS):
            for cb in range(NCB):
                keep_warm_m(3)
                phase2(js, cb)
        # drain tail: the DVE adds and out-DMAs run ~6-10us past the last
        # real matmul; keep the PE (and with it the NC clock) warm till then
        keep_warm_m(28)

    nc.compile()
    return nc


def _pack_bias(bias: np.ndarray) -> np.ndarray:
    pack = np.zeros((1, BIAS_PACK), dtype=np.float16)
    for js in range(NJS):
        for blk in range(NBLK_JS[js]):
            src0 = blk * PERIOD + js * 512
            seg = bias[src0 : src0 + 512]
            pack[0, OFF_JS[js] + blk * 512 : OFF_JS[js] + blk * 512 + len(seg)] = seg
    return pack


def kernel(x, hadamard, scale, bias):
    global LAST_RESULT
    import ml_dtypes
    from concourse.bass_utils import run_bass_kernel_spmd

    x = np.ascontiguousarray(np.asarray(x, dtype=np.float32))
    hadamard = np.asarray(hadamard, dtype=np.float32)
    bias = np.asarray(bias, dtype=np.float32)
    scale_val = float(np.asarray(scale).reshape(-1)[0])

    h2 = np.ascontiguousarray(hadamard[:, :PERIOD])
    # the whole kernel rests on the 2048-periodicity of the weight columns
    for k in range(1, NFULL):
        assert np.array_equal(hadamard[:, k * PERIOD : (k + 1) * PERIOD], h2), (
            "hadamard is not 2048-periodic; kernel assumption violated")
    assert np.array_equal(hadamard[:, NFULL * PERIOD :], h2[:, :TAIL])
    # ... and on the Sylvester block structure H_2048 = [[A, A], [A, -A]]
    A = h2[:HALF, :HALF]
    assert np.array_equal(h2[HALF:, :HALF], A)
    assert np.array_equal(h2[:HALF, HALF:], A)
    assert np.array_equal(h2[HALF:, HALF:], -A)
    Ab = A.astype(ml_dtypes.bfloat16)
    assert np.array_equal(Ab.astype(np.float32), A), "A not bf16-exact"

    key = scale_val
    if key not in _CACHE:
        _CACHE[key] = _build(scale_val)
    nc = _CACHE[key]

    bias_pack = _pack_bias(bias)
    in_maps = [
        {"x": np.ascontiguousarray(x[c * BLOC : (c + 1) * BLOC]),
         "h": Ab, "bias": bias_pack}
        for c in range(NCORES)
    ]
    res = run_bass_kernel_spmd(nc, in_maps, list(range(NCORES)),
                               trace=PROFILE)
    LAST_RESULT = res
    out = np.concatenate([res.results[c]["out"] for c in range(NCORES)],
                         axis=0).astype(np.float32)
    return out
